# revision 33
# baseline (speedup 1.0000x reference)
"""Trainium2 Bass kernel for a post-LN transformer decoder layer (v3).

Sharding: 8 cores = 4 batches x 2 token-halves (interleaved quarters:
pair-rank j=0 handles global rows [0:512)+[1536:2048), j=1 [512:1536)).

v3 vs v2 (trace-driven):
  - attention is Scalar(exp)-bound: per k-tile the PE owes 4 N=512 matmuls
    (~850ns) while ACT owes one [128,1024] exp (~1150ns).  v2 ran 4 heads
    with 1-kt-deep PSUM scoring so PE/ACT ping-ponged at ~70% each.  v3
    runs head-PAIRS in a flat software pipeline: scores double-buffered
    ([128,1024] f32 x 2 = 4 banks), u accumulators on a 4-bank rotation,
    AV lagged one unit, and ALL non-exp work off ACT (psum drains, bias
    adds, relu on DVE; softmax recip via reciprocal_approx_fast).
  - the ~30% PE slack inside attention is filled by INJECTED closures:
    during attn1 slot1: O1(qb0), LN1(qb0), V2(half0)+AllGather, Q2(hp) as
    each head pair retires; during attn2: O2(qb0), LN2(qb0), most of
    FFN1(qb0).
  - K1/K2 spill to DRAM and stream back per head pair (4KB window instead
    of 32KB resident); residuals in bf16; LN applies in the folded 2-op
    form out = (z*rs_b)*g - ((mu*rs_b)*g - be) on DVE; LN stats read bf16
    z directly.
"""

import sys

sys.path.insert(0, "/opt/trn_rl_repo")

from collections import deque

import numpy as np
import ml_dtypes

import concourse.bass as bass
import concourse.tile as tile
from concourse import mybir
from concourse.bass_utils import run_bass_kernel_spmd

BF16 = mybir.dt.bfloat16
F32 = mybir.dt.float32
AF = mybir.ActivationFunctionType
OP = mybir.AluOpType

D = 1024       # d_model
DFF = 4096
B, S = 4, 2048
NCORES = 8
QL = 1024      # local query rows per core
EPS = 1e-6
MT = 8         # d_model 128-tiles
FT = 32        # d_ff 128-tiles
SCALE = 0.125  # 1/sqrt(head_dim)
SLOT_NKT = (8, 16)  # uniform k-tile count per query slot; masks do the rest

# global key-tile t -> (pair_rank, local st) for the V2 AllGather result:
# rank j=0 owns quarters Q0,Q3 (local st 0-3 = tiles 0-3, st 4-7 = 12-15),
# rank j=1 owns Q1,Q2 (local st 0-3 = tiles 4-7, st 4-7 = tiles 8-11).
V2_SRC = ([(0, t) for t in range(4)] +
          [(1, t - 4) for t in range(4, 12)] +
          [(0, t - 8) for t in range(12, 16)])

_CACHED = {}
PHASE_MARKS = []  # (phase_name, I-number at entry) for analysis


def legalize_waits(nc, max_waits=1):
    """This walrus build accepts at most one sync-wait per instruction.
    Hoist excess waits onto same-engine NoOps inserted just before."""
    nid = 0
    for fn in nc.m.functions:
        for bb in fn.blocks:
            new = []
            changed = False
            for inst in bb.instructions:
                si = inst.sync_info
                if si is not None and si.on_wait and len(si.on_wait) > max_waits:
                    waits = list(si.on_wait)
                    for w in waits[:-max_waits]:
                        nid += 1
                        nop = mybir.InstNoOp(name=f"I-waitfix-{nid}", ins=[], outs=[])
                        nop.engine = inst.engine
                        nop.sync_info = mybir.SyncInfo(on_wait=[w], on_update=[])
                        new.append(nop)
                    inst.sync_info = mybir.SyncInfo(
                        on_wait=waits[-max_waits:], on_update=list(si.on_update)
                    )
                    changed = True
                new.append(inst)
            if changed:
                bb.instructions = new


def interleave(*lists):
    """Round-robin merge of closure lists."""
    out = []
    its = [list(l) for l in lists]
    while any(its):
        for l in its:
            if l:
                out.append(l.pop(0))
    return out


def build_nc():
    nc = bass.Bass(num_devices=NCORES)

    # -------- DRAM tensors (all host-pretiled for contiguous DMA) --------
    xT = nc.dram_tensor("xT", [128, MT, S], BF16, kind="ExternalInput")
    xTq = nc.dram_tensor("xTq", [128, MT, QL], BF16, kind="ExternalInput")
    encT = nc.dram_tensor("encT", [128, MT, S], BF16, kind="ExternalInput")
    encq = nc.dram_tensor("encq", [128, MT, QL], BF16, kind="ExternalInput")
    wd = {}
    # [p, out_tile, in_tile, 128] : slice [:, ot, :, :] is 2KB/partition
    for name in ("wq1", "wk1", "wo1", "wq2", "wk2", "wo2"):
        wd[name] = nc.dram_tensor(name, [128, MT, MT, 128], BF16,
                                  kind="ExternalInput")
    wd["wf1"] = nc.dram_tensor("wf1", [128, FT, MT, 128], BF16,
                               kind="ExternalInput")
    wd["wf2"] = nc.dram_tensor("wf2", [128, MT, FT, 128], BF16,
                               kind="ExternalInput")
    wd["wv1"] = nc.dram_tensor("wv1", [128, MT, D], BF16, kind="ExternalInput")
    wd["wv2"] = nc.dram_tensor("wv2", [128, MT, D], BF16, kind="ExternalInput")
    bias_specs = (
        ("cq1", 8), ("co1", 8), ("cq2", 8), ("co2", 8),
        ("cf1", 32), ("cf2", 8),
        ("g1", 8), ("be1", 8), ("g2", 8), ("be2", 8), ("g3", 8), ("be3", 8),
    )
    bcd = {}
    for name, k in bias_specs:
        bcd[name] = nc.dram_tensor(name, [128, k], F32, kind="ExternalInput")
    bigmask = nc.dram_tensor("bigmask", [128, 2, 8, 512], BF16,
                             kind="ExternalInput")
    outT = nc.dram_tensor("outT", [128, MT, QL], F32, kind="ExternalOutput")
    v2locH = [nc.dram_tensor(f"v2loc{i}", [128, 4 * D], BF16) for i in (0, 1)]
    v2allH = [nc.dram_tensor(f"v2all{i}", [256, 4 * D], BF16) for i in (0, 1)]
    k1T = nc.dram_tensor("k1T", [128, MT, S], BF16)   # K1 spill
    k2T = nc.dram_tensor("k2T", [128, MT, S], BF16)   # K2 spill

    with tile.TileContext(nc) as tc:
        _cms = {}

        def _mark(phase):
            nm = nc.get_next_instruction_name()  # consumes one number
            PHASE_MARKS.append((phase, int(nm.split("-")[1])))

        def open_pool(**kw):
            cm = tc.tile_pool(**kw)
            _cms[kw["name"]] = cm
            return cm.__enter__()

        def close_pool(pool_name):
            _cms.pop(pool_name).__exit__(None, None, None)

        # ---- global pools (close at the very end) ----
        const = open_pool(name="const", bufs=1)
        wp8 = open_pool(name="wp8", bufs=3)      # [128,8,128] weight tiles
        epool = open_pool(name="epool", bufs=3)  # exp tiles [128,1024] bf16
        ubp = open_pool(name="ubp", bufs=2)      # u psum->sbuf [64,512] bf16
        rowp = open_pool(name="rowp", bufs=3)    # den/rec/stat rows
        sp = open_pool(name="sp", bufs=2)        # [128,512] scratch
        psp = open_pool(name="psp", bufs=2, space="PSUM")  # s:2x2 + u:4

        # ---- left stack (chronological opens, LIFO closes) ----
        pQV = open_pool(name="pQV", bufs=1)              # close after attn2
        pKT = open_pool(name="pKT", bufs=2)              # KT stream tiles
        pKst = open_pool(name="pKst", bufs=2)            # K spill staging
        pA = open_pool(name="pA", bufs=1)                # xTs

        # ---- right stack ----
        pLNZ = open_pool(name="pLNZ", bufs=1, side="right")   # z bf16
        pX1B = open_pool(name="pX1B", bufs=1, side="right")   # x1bf
        pRES = open_pool(name="pRES", bufs=1, side="right")   # attn outs

        # startup-critical DMAs first: xTs tiles + first K1 weight tile
        xTs = pA.tile([128, MT, S], BF16, tag="xTs")
        for mt in range(MT):
            nc.sync.dma_start(out=xTs[:, mt, :], in_=xT.ap()[:, mt, :])

        bc = {}
        for name, k in bias_specs:
            t = const.tile([128, k], F32, tag=name)
            nc.sync.dma_start(out=t, in_=bcd[name][:, :])
            bc[name] = t
        ones_col = const.tile([128, 1], BF16, tag="ones_col")
        nc.vector.memset(ones_col, 1.0)
        ones_row = const.tile([1, 128], BF16, tag="ones_row")
        nc.vector.memset(ones_row, 1.0)
        epsr = const.tile([1, 1], F32, tag="epsr")
        nc.vector.memset(epsr, EPS)
        QTzG = pQV.tile([128, 16, QL], BF16, tag="QTzG")
        nc.vector.memset(QTzG, 0.0)
        VHG = pQV.tile([128, 16, 16, 65], BF16, tag="VHG")
        nc.vector.memset(VHG[:, :, :, 64:65], 1.0)

        # =============== helpers ===============
        def pp_group(wtile, src, src_cols):
            pp = psp.tile([128, 512], F32, tag="u", bufs=4)
            for mt in range(MT):
                nc.tensor.matmul(pp[:, :], wtile[:, mt, :], src[:, mt, src_cols],
                                 start=(mt == 0), stop=(mt == MT - 1))
            return pp

        def ln_closures(z, qb, gname, bename, xout):
            """z bf16 [128, MT, 512] -> xout[:, :, qb*512:+512]."""
            qs = slice(qb * 512, qb * 512 + 512)
            g = bc[gname]
            be = bc[bename]
            st = {}

            def stats_half(half):
                def fn():
                    if half == 0:
                        st["ps"] = psp.tile([1, 512], F32, tag="u", bufs=4,
                                            name="ps")
                        st["pq"] = psp.tile([1, 512], F32, tag="u", bufs=4,
                                            name="pq")
                    for mt in range(4 * half, 4 * half + 4):
                        nc.tensor.matmul(st["ps"][0:1, :], ones_col[:, :],
                                         z[:, mt, :], start=(mt == 0),
                                         stop=(mt == MT - 1))
                        zsq = sp.tile([128, 512], BF16, tag="zsq")
                        nc.vector.tensor_mul(zsq[:, :], z[:, mt, :],
                                             z[:, mt, :])
                        nc.tensor.matmul(st["pq"][0:1, :], ones_col[:, :],
                                         zsq[:, :], start=(mt == 0),
                                         stop=(mt == MT - 1))
                return fn

            def chain():
                mu = rowp.tile([1, 512], F32, tag="r32")
                nc.vector.tensor_scalar(mu[:, :], st["ps"][0:1, :], 1.0 / D,
                                        None, op0=OP.mult)
                t = rowp.tile([1, 512], F32, tag="r32")
                nc.vector.tensor_scalar(t[:, :], st["pq"][0:1, :], 1.0 / D,
                                        None, op0=OP.mult)
                musq = rowp.tile([1, 512], F32, tag="r32")
                nc.vector.tensor_mul(musq[:, :], mu[:, :], mu[:, :])
                nc.vector.tensor_sub(t[:, :], t[:, :], musq[:, :])
                nc.scalar.activation(t[:, :], t[:, :], AF.Sqrt, bias=epsr[:, :])
                rstd = rowp.tile([1, 512], F32, tag="r32b", bufs=2)
                nc.vector.reciprocal(rstd[:, :], t[:, :])
                murs = rowp.tile([1, 512], F32, tag="r32b", bufs=2)
                nc.vector.tensor_mul(murs[:, :], mu[:, :], rstd[:, :])
                rsb = rowp.tile([1, 512], BF16, tag="rb16")
                nc.vector.tensor_copy(rsb[:, :], rstd[:, :])
                mursb = rowp.tile([1, 512], BF16, tag="rb16")
                nc.vector.tensor_copy(mursb[:, :], murs[:, :])
                st["rsb"], st["mursb"] = rsb, mursb

            def bcast():
                rs_p = psp.tile([128, 512], F32, tag="u", bufs=4, name="rsp")
                nc.tensor.matmul(rs_p[:, :], ones_row[:, :], st["rsb"][:, :],
                                 start=True, stop=True)
                m_p = psp.tile([128, 512], F32, tag="u", bufs=4, name="mp")
                nc.tensor.matmul(m_p[:, :], ones_row[:, :], st["mursb"][:, :],
                                 start=True, stop=True)
                rs_b = sp.tile([128, 512], F32, tag="rsbs", bufs=1)
                nc.vector.tensor_copy(rs_b[:, :], rs_p[:, :])
                m_b = sp.tile([128, 512], F32, tag="mbs", bufs=1)
                nc.vector.tensor_copy(m_b[:, :], m_p[:, :])
                st["rs_b"], st["m_b"] = rs_b, m_b

            def apply_pair(m0):
                def fn():
                    for mt in (m0, m0 + 1):
                        zA = sp.tile([128, 512], BF16, tag="zA")
                        nc.vector.tensor_mul(zA[:, :], z[:, mt, :],
                                             st["rs_b"][:, :])
                        bnegm = sp.tile([128, 512], BF16, tag="bneg")
                        nc.vector.tensor_scalar(
                            bnegm[:, :], st["m_b"][:, :], g[:, mt:mt + 1],
                            be[:, mt:mt + 1], op0=OP.mult, op1=OP.subtract)
                        nc.vector.scalar_tensor_tensor(
                            xout[:, mt, qs], zA[:, :], g[:, mt:mt + 1],
                            bnegm[:, :], op0=OP.mult, op1=OP.subtract)
                return fn

            cls = [stats_half(0), stats_half(1), chain, bcast]
            for m0 in range(0, MT, 2):
                cls.append(apply_pair(m0))
            return cls

        # =============== KT streaming ===============
        def make_kt_stream(dram):
            seq = [(slot, hp) for slot in (0, 1) for hp in range(8)]
            tiles = {}

            def ensure(idx):
                if idx >= len(seq) or seq[idx] in tiles:
                    return
                t = pKT.tile([128, S], BF16, tag="kt", name="ktile", bufs=2)
                nc.sync.dma_start(out=t, in_=dram.ap()[:, seq[idx][1], :])
                tiles[seq[idx]] = t

            def get(slot, hp):
                idx = seq.index((slot, hp))
                ensure(idx)
                ensure(idx + 1)
                return tiles.pop((slot, hp))
            ensure(0)
            return get

        # =============== attention phase ===============
        def attention_phase(kt_get, VH, QTz, attnout, nkts, maskts, inject,
                            retire_hook=None):
            inject = deque(inject)
            fins = deque()
            pend = [None]
            ui = [0]

            def end_pair(p):
                qs = p["qs"]
                hp = p["hp"]
                ubs = []
                den2 = rowp.tile([65, 512], F32, tag="den", bufs=2, name="den2")
                for i in range(2):
                    u = p["us"][i]
                    ub = ubp.tile([64, 512], BF16, tag="ub")
                    nc.vector.tensor_copy(ub[:, :], u[0:64, :])
                    nc.vector.tensor_copy(den2[64 * i:64 * i + 1, :],
                                          u[64:65, :])
                    ubs.append(ub)
                recbh = {}

                rech = {}

                def mk_rchunk(c):
                    def fr():
                        if c == 0:
                            rech[0] = rowp.tile([65, 512], F32, tag="den",
                                                bufs=2, name="rec2")
                        nc.vector.reciprocal(rech[0][:, c:c + 128],
                                             den2[:, c:c + 128])
                    return fr

                def f1():
                    rec = rech[0]
                    for i in range(2):
                        rcb = rowp.tile([1, 512], BF16, tag="rb16",
                                        name="recb")
                        nc.vector.tensor_copy(rcb[:, :],
                                              rec[64 * i:64 * i + 1, :])
                        recbh[i] = rcb

                def mk2(i):
                    def f2():
                        recb = recbh[i]
                        rb = psp.tile([64, 512], F32, tag="u", bufs=4,
                                      name="rb")
                        nc.tensor.matmul(rb[:, :], ones_row[:, 0:64],
                                         recb[:, :], start=True,
                                         stop=True)
                        hs0 = i * 64
                        nc.vector.tensor_mul(attnout[hs0:hs0 + 64, hp, qs],
                                             ubs[i][:, :], rb[:, :])
                    return f2

                for c in range(0, 512, 128):
                    fins.append(mk_rchunk(c))
                fins.append(f1)
                fins.append(mk2(0))
                fins.append(mk2(1))
                if retire_hook is not None:
                    retire_hook(hp, p["slot"], inject, ui[0])

            def flush_pend():
                p = pend[0]
                if p is None:
                    return
                pend[0] = None
                h0, h1 = 2 * p["hp"], 2 * p["hp"] + 1
                eb = p["eblk"]
                nc.tensor.matmul(p["us"][0][:, :], VH[:, p["kt"], h0, :],
                                 eb[:, 0:512], start=(p["kt"] == 0),
                                 stop=p["last"])
                nc.tensor.matmul(p["us"][1][:, :], VH[:, p["kt"], h1, :],
                                 eb[:, 512:1024], start=(p["kt"] == 0),
                                 stop=p["last"])
                if p["last"]:
                    end_pair(p)

            for slot in range(2):
                nkt = nkts[slot]
                qs = slice(slot * 512, slot * 512 + 512)
                maskt = maskts(slot) if maskts is not None else None
                for hp in range(8):
                    ktile = kt_get(slot, hp)
                    us = None
                    for kt in range(nkt):
                        if kt == 0:
                            us = (psp.tile([65, 512], F32, tag="u", bufs=4,
                                           name="u0"),
                                  psp.tile([65, 512], F32, tag="u", bufs=4,
                                           name="u1"))
                        sblk = psp.tile([128, 1024], F32, tag="s", bufs=2)
                        ks = slice(kt * 128, kt * 128 + 128)
                        nc.tensor.matmul(sblk[:, 0:512], ktile[:, ks],
                                         QTz[:, 2 * hp, qs], start=True,
                                         stop=True)
                        nc.tensor.matmul(sblk[:, 512:1024], ktile[:, ks],
                                         QTz[:, 2 * hp + 1, qs], start=True,
                                         stop=True)
                        eblk = epool.tile([128, 1024], BF16, tag="e", bufs=3)
                        nc.scalar.activation(eblk[:, :], sblk[:, :], AF.Exp,
                                             scale=SCALE)
                        if maskt is not None and kt >= nkt - 8:
                            ki = kt - (nkt - 8)
                            nc.vector.tensor_mul(eblk[:, 0:512],
                                                 eblk[:, 0:512],
                                                 maskt[:, ki, :])
                            nc.vector.tensor_mul(eblk[:, 512:1024],
                                                 eblk[:, 512:1024],
                                                 maskt[:, ki, :])
                        if fins:
                            fins.popleft()()
                        flush_pend()
                        pend[0] = dict(eblk=eblk, hp=hp, kt=kt, slot=slot,
                                       last=(kt == nkt - 1), us=us, qs=qs)
                        if inject and ui[0] >= inject[0][0]:
                            inject.popleft()[1]()
                        ui[0] += 1
            flush_pend()
            while fins:
                fins.popleft()()
            while inject:
                inject.popleft()[1]()

        # ================= PHASE A =================
        _mark("K1")
        for hp in range(MT):
            wt = wp8.tile([128, MT, 128], BF16, tag="w8")
            nc.sync.dma_start(out=wt, in_=wd["wk1"].ap()[:, hp, :, :])
            kst = pKst.tile([128, S], BF16, tag="kst1", bufs=1)
            if hp == 0:
                # mt-outer: first matmul only needs xTs tile 0 (starts
                # ~1.8us in, instead of waiting for the full 32KB load)
                pps = [psp.tile([128, 512], F32, tag="u", bufs=4,
                                name=f"ppk1{sb}") for sb in range(4)]
                for mt in range(MT):
                    for sb in range(4):
                        ss = slice(sb * 512, sb * 512 + 512)
                        nc.tensor.matmul(pps[sb][:, :], wt[:, mt, :],
                                         xTs[:, mt, ss],
                                         start=(mt == 0),
                                         stop=(mt == MT - 1))
                for sb in range(4):
                    ss = slice(sb * 512, sb * 512 + 512)
                    nc.vector.tensor_copy(kst[:, ss], pps[sb][:, :])
            else:
                for sb in range(4):
                    ss = slice(sb * 512, sb * 512 + 512)
                    pp = pp_group(wt, xTs, ss)
                    nc.vector.tensor_copy(kst[:, ss], pp[:, :])
            nc.sync.dma_start(out=k1T.ap()[:, hp, :], in_=kst)
        _mark("V1")
        wpvA = open_pool(name="wpvA", bufs=1)
        for vb in range(2):
            dsl = slice(vb * 512, vb * 512 + 512)
            wv1s = wpvA.tile([128, MT, 512], BF16, tag="wv")
            nc.sync.dma_start(out=wv1s, in_=wd["wv1"].ap()[:, :, dsl])
            for st_ in range(16):
                ts_ = slice(st_ * 128, st_ * 128 + 128)
                pp = psp.tile([128, 512], F32, tag="u", bufs=4)
                for mt in range(MT):
                    nc.tensor.matmul(pp[:, :], xTs[:, mt, ts_],
                                     wv1s[:, mt, :],
                                     start=(mt == 0), stop=(mt == MT - 1))
                nc.vector.tensor_copy(
                    VHG[:, st_, vb * 8:vb * 8 + 8, 0:64],
                    pp[:, :].rearrange("p (h d) -> p h d", h=8))

        close_pool("wpvA")
        close_pool("pA")
        _mark("Q1")
        pAq = open_pool(name="pAq", bufs=1)              # xqs (residual 1)
        xqs = pAq.tile([128, MT, QL], BF16, tag="xqs")
        for mt in range(MT):
            nc.sync.dma_start(out=xqs[:, mt, :], in_=xTq.ap()[:, mt, :])
        for hp in range(MT):
            wt = wp8.tile([128, MT, 128], BF16, tag="w8")
            nc.sync.dma_start(out=wt, in_=wd["wq1"].ap()[:, hp, :, :])
            for qb in range(2):
                qs = slice(qb * 512, qb * 512 + 512)
                pp = pp_group(wt, xqs, qs)
                nc.scalar.activation(QTzG[0:64, 2 * hp, qs], pp[0:64, :],
                                     AF.Identity,
                                     bias=bc["cq1"][0:64, hp:hp + 1])
                nc.scalar.activation(QTzG[64:128, 2 * hp + 1, qs],
                                     pp[64:128, :], AF.Identity,
                                     bias=bc["cq1"][64:128, hp:hp + 1])
        close_pool("pAq")

        # encqs for Q2 (used inside attn1), wv2 for V2
        pAq2 = open_pool(name="pAq2", bufs=1)
        encqs = pAq2.tile([128, MT, QL], BF16, tag="encqs")
        for mt in range(MT):
            nc.sync.dma_start(out=encqs[:, mt, :], in_=encq.ap()[:, mt, :])
        wv2s_h = {}   # filled at the boundary (wpvB pool)

        # masks: single 8KB buffer, reloaded per slot
        pMSK = open_pool(name="pMSK", bufs=1)
        mask_tiles = {}

        def mask_get(slot):
            m = pMSK.tile([128, 8, 512], BF16, tag="mask")
            nc.sync.dma_start(out=m, in_=bigmask.ap()[:, slot, :, :])
            mask_tiles[slot] = m
            return m

        # ---- buffers for the attn1-injected qb0 chain ----
        attn1o = pRES.tile([128, MT, QL], BF16, tag="attno")
        x1bf = pX1B.tile([128, MT, QL], BF16, tag="x1bf")

        def o_proj_closures(wname, biasname, attno, resid, ztile, qb,
                            resid_dram=None):
            qs = slice(qb * 512, qb * 512 + 512)
            wts = {}
            pps = {}
            rts = {}

            def mk(nt, half):
                def fn():
                    if half == 0:
                        if nt == 0:
                            wts[0] = wp8.tile([128, MT, 128], BF16, tag="w8",
                                              name="w8a")
                            nc.sync.dma_start(out=wts[0],
                                              in_=wd[wname].ap()[:, 0, :, :])
                        if nt + 1 < MT:
                            wts[nt + 1] = wp8.tile([128, MT, 128], BF16,
                                                   tag="w8", name="w8b")
                            nc.sync.dma_start(
                                out=wts[nt + 1],
                                in_=wd[wname].ap()[:, nt + 1, :, :])
                        if resid_dram is not None:
                            rts[nt] = sp.tile([128, 512], BF16, tag="resd",
                                              bufs=2, name="resd")
                            nc.sync.dma_start(out=rts[nt],
                                              in_=resid_dram.ap()[:, nt, qs])
                        pps[nt] = psp.tile([128, 512], F32, tag="u", bufs=4, name="ppo")
                        for dt in range(4):
                            nc.tensor.matmul(pps[nt][:, :],
                                             wts[nt][:, dt, :],
                                             attno[:, dt, qs],
                                             start=(dt == 0), stop=False)
                    else:
                        wt = wts.pop(nt)
                        pp = pps.pop(nt)
                        for dt in range(4, MT):
                            nc.tensor.matmul(pp[:, :], wt[:, dt, :],
                                             attno[:, dt, qs],
                                             start=False, stop=(dt == MT - 1))
                        rs = (rts.pop(nt)[:, :] if resid_dram is not None
                              else resid[:, nt, qs])
                        nc.vector.scalar_tensor_tensor(
                            ztile[:, nt, :], pp[:, :],
                            bc[biasname][:, nt:nt + 1],
                            rs, op0=OP.add, op1=OP.add)
                return fn
            return [mk(nt, h) for nt in range(MT) for h in (0, 1)]

        def v2_closures(i):
            cls = []

            pph = {}

            def mk(st_, vb, half, holder):
                def fn():
                    ts_ = slice(st_ * 128, st_ * 128 + 128)
                    dsl = slice(vb * 512, vb * 512 + 512)
                    if half == 0:
                        if vb == 0:
                            holder[0] = pV2.tile([128, D], BF16, tag="v2sb",
                                                 bufs=2, name="v2sb")
                        pph[(st_, vb)] = psp.tile([128, 512], F32, tag="u",
                                                  bufs=4, name="ppv2")
                        for mt in range(4):
                            nc.tensor.matmul(pph[(st_, vb)][:, :],
                                             x1bf[:, mt, ts_],
                                             wv2s_h[0][:, mt, dsl],
                                             start=(mt == 0), stop=False)
                    else:
                        pp = pph.pop((st_, vb))
                        for mt in range(4, MT):
                            nc.tensor.matmul(pp[:, :], x1bf[:, mt, ts_],
                                             wv2s_h[0][:, mt, dsl],
                                             start=False, stop=(mt == MT - 1))
                        nc.vector.tensor_copy(holder[0][:, dsl], pp[:, :])
                        if vb == 1:
                            nc.sync.dma_start(
                                out=v2locH[i].ap().rearrange(
                                    "p (st d) -> p st d",
                                    st=4)[:, st_ - 4 * i, :],
                                in_=holder[0])
                return fn
            for st_ in range(4 * i, 4 * i + 4):
                holder = [None]
                for vb in (0, 1):
                    cls.append(mk(st_, vb, 0, holder))
                    cls.append(mk(st_, vb, 1, holder))

            def gather():
                nc.gpsimd.collective_compute(
                    "AllGather",
                    mybir.AluOpType.bypass,
                    replica_groups=[[2 * p, 2 * p + 1] for p in range(4)],
                    ins=[v2locH[i][:, :]],
                    outs=[v2allH[i][:, :]],
                )
            cls.append(gather)
            return cls

        def q2_emit(hp):
            st2 = {}

            def mk(qb, half):
                def fn():
                    qs = slice(qb * 512, qb * 512 + 512)
                    if half == 0:
                        st2[qb] = (wp8.tile([128, MT, 128], BF16, tag="w8",
                                            name="w8q2"),
                                   psp.tile([128, 512], F32, tag="u", bufs=4,
                                            name="ppq2"))
                        wt, pp = st2[qb]
                        nc.sync.dma_start(out=wt,
                                          in_=wd["wq2"].ap()[:, hp, :, :])
                        for mt in range(4):
                            nc.tensor.matmul(pp[:, :], wt[:, mt, :],
                                             encqs[:, mt, qs],
                                             start=(mt == 0), stop=False)
                    else:
                        wt, pp = st2.pop(qb)
                        for mt in range(4, MT):
                            nc.tensor.matmul(pp[:, :], wt[:, mt, :],
                                             encqs[:, mt, qs],
                                             start=False, stop=(mt == MT - 1))
                        nc.vector.tensor_scalar(
                            QTzG[0:64, 2 * hp, qs], pp[0:64, :],
                            bc["cq2"][0:64, hp:hp + 1], None, op0=OP.add)
                        nc.vector.tensor_scalar(
                            QTzG[64:128, 2 * hp + 1, qs], pp[64:128, :],
                            bc["cq2"][64:128, hp:hp + 1], None, op0=OP.add)
                return fn
            return [mk(0, 0), mk(0, 1), mk(1, 0), mk(1, 1)]

        def q2_retire_hook(hp, slot, inject_q, ui_now):
            if slot != 1 or hp >= 7:
                return
            for fn in q2_emit(hp):
                inject_q.append([ui_now, fn])

        z1t0 = pLNZ.tile([128, MT, 512], BF16, tag="z")
        inj1 = []
        o1c = o_proj_closures("wo1", "co1", attn1o, None, z1t0, 0,
                              resid_dram=xTq)
        for k, fn in enumerate(o1c):
            inj1.append([64 + 6 + 2 * k, fn])
        for k, fn in enumerate(ln_closures(z1t0, 0, "g1", "be1", x1bf)):
            inj1.append([64 + 42 + 3 * k, fn])

        _mark("attn1")
        kt1_get = make_kt_stream(k1T)
        attention_phase(kt1_get, VHG, QTzG, attn1o, SLOT_NKT, mask_get, inj1,
                        retire_hook=q2_retire_hook)
        close_pool("pMSK")

        # ================= boundary =================
        _mark("bnd")
        pV2 = open_pool(name="pV2", bufs=2)              # v2 staging
        wpvB = open_pool(name="wpvB", bufs=1)
        wv2s_h[0] = wpvB.tile([128, MT, D], BF16, tag="wv", name="wv2s")
        nc.sync.dma_start(out=wv2s_h[0], in_=wd["wv2"].ap())
        pB = open_pool(name="pB", bufs=1)

        def vh2_build(i):
            for t in range(16):
                rank, stl = V2_SRC[t]
                if stl // 4 != i:
                    continue
                v2st = pV2.tile([128, D], BF16, tag="v2sb", bufs=2,
                                name="v2st")
                nc.sync.dma_start(
                    out=v2st,
                    in_=v2allH[i][rank * 128:rank * 128 + 128,
                                  (stl - 4 * i) * D:(stl - 4 * i) * D + D])
                nc.vector.tensor_copy(
                    VHG[:, t, :, 0:64],
                    v2st.rearrange("p (h d) -> p h d", h=16))

        def k2_closures():
            cls = []
            encs_h = {}
            sth = {}

            def load_encs(half, sb):
                def fn():
                    c0 = half * QL + sb * 512
                    encs_h[(half, sb)] = pB.tile([128, MT, 512], BF16,
                                                 tag="encs", name="encs")
                    for mt in range(MT):
                        nc.sync.dma_start(
                            out=encs_h[(half, sb)][:, mt, :],
                            in_=encT.ap()[:, mt, c0:c0 + 512])
                return fn

            def mk(half, hp, sb, h2):
                def fn():
                    key = (half, sb, hp)
                    if h2 == 0:
                        if hp == 0:
                            sth[(half, sb, 0, "w")] = wp8.tile(
                                [128, MT, 128], BF16, tag="w8", name="w8k2")
                            nc.sync.dma_start(
                                out=sth[(half, sb, 0, "w")],
                                in_=wd["wk2"].ap()[:, 0, :, :])
                        if hp + 1 < MT:  # prefetch next hp's tile
                            sth[(half, sb, hp + 1, "w")] = wp8.tile(
                                [128, MT, 128], BF16, tag="w8", name="w8k2b")
                            nc.sync.dma_start(
                                out=sth[(half, sb, hp + 1, "w")],
                                in_=wd["wk2"].ap()[:, hp + 1, :, :])
                        ppk = psp.tile([128, 512], F32, tag="u", bufs=4,
                                       name="ppk2")
                        sth[key] = ppk
                        wt = sth[(half, sb, hp, "w")]
                        encs = encs_h[(half, sb)]
                        for mt in range(4):
                            nc.tensor.matmul(ppk[:, :], wt[:, mt, :],
                                             encs[:, mt, :],
                                             start=(mt == 0), stop=False)
                    else:
                        pp = sth.pop(key)
                        wt = sth.pop((half, sb, hp, "w"))
                        encs = encs_h[(half, sb)]
                        for mt in range(4, MT):
                            nc.tensor.matmul(pp[:, :], wt[:, mt, :],
                                             encs[:, mt, :],
                                             start=False, stop=(mt == MT - 1))
                        kst = pKst.tile([128, 512], BF16, tag="kst2",
                                        bufs=2, name="kst2")
                        nc.vector.tensor_copy(kst[:, :], pp[:, :])
                        c0 = half * QL + sb * 512
                        nc.sync.dma_start(
                            out=k2T.ap()[:, hp, c0:c0 + 512], in_=kst)
                return fn

            # sb-major so each 8KB encs chunk serves 8 hp groups before
            # the single buffer rotates
            for half in range(2):
                for sb in range(2):
                    cls.append(load_encs(half, sb))
                    for hp in range(MT):
                        for h2 in (0, 1):
                            cls.append(mk(half, hp, sb, h2))
            return cls

        z1t1 = pLNZ.tile([128, MT, 512], BF16, tag="z")
        chain = (q2_emit(7) +
                 o_proj_closures("wo1", "co1", attn1o, None, z1t1, 1,
                                 resid_dram=xTq) +
                 ln_closures(z1t1, 1, "g1", "be1", x1bf) +
                 v2_closures(0) + v2_closures(1))
        for fn in interleave(k2_closures(), chain):
            fn()
        vh2_build(0)
        vh2_build(1)
        close_pool("pB")
        close_pool("wpvB")
        close_pool("pV2")
        close_pool("pAq2")
        close_pool("pKst")

        # ================= attn2 =================
        attn2o = pRES.tile([128, MT, QL], BF16, tag="attno")
        pX2B = open_pool(name="pX2B", bufs=1, side="right")
        x2bf = pX2B.tile([128, MT, QL], BF16, tag="x2bf")
        pHT = open_pool(name="pHT", bufs=1, side="right")
        hT = {0: pHT.tile([128, FT, 512], BF16, tag="hT", name="hT0")}

        z2t0 = pLNZ.tile([128, MT, 512], BF16, tag="z")

        def ffn1_closures(qb, on_act):
            qs = slice(qb * 512, qb * 512 + 512)
            wts = {}

            pps = {}

            def mk(ft, half):
                def fn():
                    if half == 0:
                        if ft == 0:
                            wts[0] = wp8.tile([128, MT, 128], BF16, tag="w8",
                                              name="w8f")
                            nc.sync.dma_start(out=wts[0],
                                              in_=wd["wf1"].ap()[:, 0, :, :])
                        if ft + 1 < FT:
                            wts[ft + 1] = wp8.tile([128, MT, 128], BF16,
                                                   tag="w8", name="w8g")
                            nc.sync.dma_start(
                                out=wts[ft + 1],
                                in_=wd["wf1"].ap()[:, ft + 1, :, :])
                        pps[ft] = psp.tile([128, 512], F32, tag="u", bufs=4, name="ppf")
                        for mt in range(4):
                            nc.tensor.matmul(pps[ft][:, :], wts[ft][:, mt, :],
                                             x2bf[:, mt, qs],
                                             start=(mt == 0), stop=False)
                    else:
                        wt = wts.pop(ft)
                        pp = pps.pop(ft)
                        for mt in range(4, MT):
                            nc.tensor.matmul(pp[:, :], wt[:, mt, :],
                                             x2bf[:, mt, qs],
                                             start=False, stop=(mt == MT - 1))
                        if on_act:
                            nc.scalar.activation(hT[qb][:, ft, :], pp[:, :],
                                                 AF.Relu,
                                                 bias=bc["cf1"][:, ft:ft + 1])
                        else:
                            nc.vector.tensor_scalar(
                                hT[qb][:, ft, :], pp[:, :],
                                bc["cf1"][:, ft:ft + 1], 0.0,
                                op0=OP.add, op1=OP.max)
                return fn
            return [mk(ft, h) for ft in range(FT) for h in (0, 1)]

        inj2 = []
        o2c = o_proj_closures("wo2", "co2", attn2o, x1bf, z2t0, 0)
        for k, fn in enumerate(o2c):
            inj2.append([128 + 6 + 2 * k, fn])
        for k, fn in enumerate(ln_closures(z2t0, 0, "g2", "be2", x2bf)):
            inj2.append([128 + 42 + 3 * k, fn])
        ffn0 = ffn1_closures(0, on_act=False)
        for k, fn in enumerate(ffn0[:56]):
            inj2.append([128 + 68 + 1 * k, fn])
        ffn0_rest = ffn0[56:]

        _mark("attn2")
        kt2_get = make_kt_stream(k2T)
        attention_phase(kt2_get, VHG, QTzG, attn2o, (16, 16), None, inj2)
        close_pool("pKT")
        close_pool("pQV")

        # ================= tail =================
        _mark("tail")
        ffn2_pool = open_pool(name="pF2", bufs=2)
        pOUT = open_pool(name="pOUT", bufs=1)

        def ffn2_closures(qb, ztile):
            qs = slice(qb * 512, qb * 512 + 512)
            wts = {}

            def mk(nt):
                def fn():
                    if nt == 0:
                        wts[0] = ffn2_pool.tile([128, FT, 128], BF16,
                                                tag="wf2t", name="wf2a")
                        nc.sync.dma_start(out=wts[0],
                                          in_=wd["wf2"].ap()[:, 0, :, :])
                    if nt + 1 < MT:
                        wts[nt + 1] = ffn2_pool.tile([128, FT, 128], BF16,
                                                     tag="wf2t", name="wf2b")
                        nc.sync.dma_start(out=wts[nt + 1],
                                          in_=wd["wf2"].ap()[:, nt + 1, :, :])
                    wt2 = wts.pop(nt)
                    pp = psp.tile([128, 512], F32, tag="u", bufs=4)
                    for ft in range(FT):
                        nc.tensor.matmul(pp[:, :], wt2[:, ft, :],
                                         hT[qb][:, ft, :],
                                         start=(ft == 0), stop=(ft == FT - 1))
                    nc.vector.scalar_tensor_tensor(
                        ztile[:, nt, :], pp[:, :], bc["cf2"][:, nt:nt + 1],
                        x2bf[:, nt, qs], op0=OP.add, op1=OP.add)
                return fn
            return [mk(nt) for nt in range(MT)]

        # step 1: finish FFN1 qb0 interleaved with O2 qb1 + LN2 qb1
        z2t1 = pLNZ.tile([128, MT, 512], BF16, tag="z")
        listA = (o_proj_closures("wo2", "co2", attn2o, x1bf, z2t1, 1) +
                 ln_closures(z2t1, 1, "g2", "be2", x2bf))
        for fn in interleave(list(ffn0_rest), listA):
            fn()

        # step 2a: FFN2 qb0 + LN3 qb0 (hT slot must free before FFN1 qb1)
        z3t0 = pLNZ.tile([128, MT, 512], BF16, tag="z")
        outsb0 = pOUT.tile([128, MT, 512], F32, tag="outsb")
        for fn in (ffn2_closures(0, z3t0) +
                   ln_closures(z3t0, 0, "g3", "be3", outsb0)):
            fn()
        nc.sync.dma_start(out=outT.ap()[:, :, 0:512], in_=outsb0)

        # step 2b: FFN1 qb1 (reuses the hT slot)
        hT[1] = pHT.tile([128, FT, 512], BF16, tag="hT", name="hT1")
        for fn in ffn1_closures(1, on_act=True):
            fn()

        # step 3: FFN2 qb1 + LN3 qb1
        z3t1 = pLNZ.tile([128, MT, 512], BF16, tag="z")
        outsb1 = pOUT.tile([128, MT, 512], F32, tag="outsb")
        for fn in (ffn2_closures(1, z3t1) +
                   ln_closures(z3t1, 0, "g3", "be3", outsb1)):
            fn()
        nc.sync.dma_start(out=outT.ap()[:, :, 512:1024], in_=outsb1)

        close_pool("pOUT")
        close_pool("pF2")
        _mark("zzEND")

        close_pool("pHT")
        close_pool("pX2B")
        close_pool("pRES")
        close_pool("pX1B")
        close_pool("pLNZ")

        for nm in ("psp", "sp", "rowp", "ubp", "epool", "wp8", "const"):
            close_pool(nm)

    return nc


def _get_nc():
    if "nc" not in _CACHED:
        nc = build_nc()
        legalize_waits(nc)
        _CACHED["nc"] = nc
    return _CACHED["nc"]


def _colbias(v, k=8):
    return np.ascontiguousarray(np.asarray(v, np.float32).reshape(k, 128).T)


def _bf(a):
    return np.ascontiguousarray(np.asarray(a)).astype(ml_dtypes.bfloat16)


def _tile_w(w, n_in_t, n_out_t):
    """[n_in_t*128, n_out_t*128] -> [128, n_out_t, n_in_t, 128] bf16."""
    a = np.asarray(w, np.float32).reshape(n_in_t, 128, n_out_t, 128)
    return _bf(a.transpose(1, 2, 0, 3))


def _tile_xT(xb):
    """x [S?, D] -> transposed tiled [128, MT, S]: element (p, mt, s) =
    x[s, mt*128+p]."""
    a = np.ascontiguousarray(np.asarray(xb, np.float32).T)  # [D, S]
    return a.reshape(MT, 128, -1).transpose(1, 0, 2)


def _make_mask(j):
    q0s = (0, 1536) if j == 0 else (512, 1024)
    m = np.zeros((2, 8, 128, 512), np.float32)
    for sl in range(2):
        q0 = q0s[sl]
        for ki in range(8):
            kt = ki if sl == 0 else 8 + ki
            k0 = kt * 128
            i = np.arange(128)[:, None]
            jq = np.arange(512)[None, :]
            m[sl, ki] = ((q0 + jq) >= (k0 + i)).astype(np.float32)
    # -> [128, 2, 8, 512]
    return _bf(m.transpose(2, 0, 1, 3))


def kernel(**inputs):
    x = np.asarray(inputs["x"], np.float32)
    enc = np.asarray(inputs["encoder_output"], np.float32)
    f32 = lambda k: np.asarray(inputs[k], np.float32)
    shared = {}
    for name in ("wq1", "wk1", "wo1", "wq2", "wk2", "wo2"):
        shared[name] = np.ascontiguousarray(_tile_w(inputs[name], MT, MT))
    shared["wf1"] = np.ascontiguousarray(_tile_w(inputs["wf1"], MT, FT))
    shared["wf2"] = np.ascontiguousarray(_tile_w(inputs["wf2"], FT, MT))
    for name in ("wv1", "wv2"):
        a = f32(name).reshape(MT, 128, D)
        shared[name] = _bf(a.transpose(1, 0, 2))
    # fold V bias into out-proj bias: out = wo.T @ (attn + bv) + bo
    co1 = f32("bo1") + f32("bv1") @ f32("wo1")
    co2 = f32("bo2") + f32("bv2") @ f32("wo2")
    shared["co1"] = _colbias(co1, 8)
    shared["co2"] = _colbias(co2, 8)
    for src, dst in (("bq1", "cq1"), ("bq2", "cq2"),
                     ("g1", "g1"), ("be1", "be1"), ("g2", "g2"),
                     ("be2", "be2"), ("g3", "g3"), ("be3", "be3")):
        shared[dst] = _colbias(inputs[src], 8)
    shared["cf1"] = _colbias(inputs["bf1"], 32)
    shared["cf2"] = _colbias(inputs["bf2"], 8)
    masks = {0: _make_mask(0), 1: _make_mask(1)}

    in_maps = []
    col_list = []
    for c in range(NCORES):
        b, j = c // 2, c % 2
        q0a, q0b = (0, 1536) if j == 0 else (512, 1024)
        cols = np.r_[q0a:q0a + 512, q0b:q0b + 512]
        col_list.append((b, cols))
        xt_t = _tile_xT(x[b])           # [128, MT, S] f32
        enc_t = _tile_xT(enc[b])
        m = dict(shared)
        m["xT"] = _bf(xt_t)
        m["xTq"] = _bf(xt_t[:, :, cols])
        m["encT"] = _bf(enc_t)
        m["encq"] = _bf(enc_t[:, :, cols])
        m["bigmask"] = masks[j]
        in_maps.append(m)

    global _LAST_IN_MAPS
    _LAST_IN_MAPS = in_maps
    nc = _get_nc()
    res = run_bass_kernel_spmd(nc, in_maps, core_ids=list(range(NCORES)))
    out = np.empty((B, S, D), np.float32)
    for c in range(NCORES):
        b, cols = col_list[c]
        o = res.results[c]["outT"]        # [128, MT, QL]
        out[b, cols, :] = o.transpose(2, 1, 0).reshape(QL, D)
    return out
